# revision 6
# baseline (speedup 1.0000x reference)
"""GCN message-passing kernel for 8 trn2 NeuronCores (bass/Tile).

Sharding: nodes are degree-sorted and dealt round-robin across 8 cores
(graph-parallel, dst-sharded).  Each core computes h@W for its own node
shard, chunked AllGathers replicate the scaled table, and each core then
aggregates its own destination nodes with per-edge indirect-DMA gathers
followed by an on-chip weighted tree-fold reduction on DVE (weights are
broadcast from SBUF via paired bf16 APs; no per-edge weight stream from
HBM).  Small 128x128 weights are replicated; BN (eval mode) is folded
into the weights/bias on device.
"""

import sys

sys.path.insert(0, "/opt/trn_rl_repo")

import numpy as np
import ml_dtypes

import concourse.bass as bass
import concourse.bacc as bacc
import concourse.mybir as mybir
from concourse.bass_utils import run_bass_kernel_spmd
from concourse.masks import make_identity
from concourse.tile import TileContext

N = 50000
E = 800000
CIN = 128
CH = 128
COUT = 64
EPS = 1e-5
NCORES = 8
P = 128

F32 = mybir.dt.float32
BF16 = mybir.dt.bfloat16
I32 = mybir.dt.int32

SHARD_N = 6272          # 49 blocks * 128
NBLOCKS = SHARD_N // P  # 49
# AllGather chunk boundaries, in blocks
CHUNK_BLOCKS = [(0, 13), (13, 25), (25, 37), (37, 49)]


def _host_prep(x, edge_index, edge_weights):
    """Pure index/layout work: shard nodes, build per-core slot layout."""
    src = edge_index[0].astype(np.int64)
    dst = edge_index[1].astype(np.int64)
    ew = edge_weights.astype(np.float32)

    deg = np.bincount(dst, minlength=N)

    order = np.argsort(deg, kind="stable")  # node_of_rank
    rank_of_node = np.empty(N, np.int64)
    rank_of_node[order] = np.arange(N)

    shard_n = SHARD_N
    nblocks = NBLOCKS
    tj = np.zeros(nblocks, np.int64)
    degs_by_rank = deg[order]
    for j in range(nblocks):
        lo, hi = j * 1024, min((j + 1) * 1024, N)
        tj[j] = degs_by_rank[lo:hi].max() if lo < N else 1
    colbase = np.concatenate([[0], np.cumsum(tj)])
    S = int(colbase[-1])

    # table row of node n (AllGather layout: [core0 shard | core1 shard | ...])
    r = rank_of_node
    table_row = (r % NCORES) * shard_n + (r // NCORES)

    idx_arr = np.zeros((NCORES, P, S), np.int32)
    w_arr = np.zeros((NCORES, P, S), np.float32)

    dr = rank_of_node[dst]
    e_order = np.argsort(dr, kind="stable")
    dr_s = dr[e_order]
    src_s = src[e_order]
    ew_s = ew[e_order]
    starts = np.searchsorted(dr_s, np.arange(N))
    slot = np.arange(len(dr_s)) - starts[dr_s]

    core = dr_s % NCORES
    local = dr_s // NCORES
    block = local // P
    part = local % P
    col = colbase[block] + slot
    idx_arr[core, part, col] = table_row[src_s].astype(np.int32)
    w_arr[core, part, col] = ew_s

    # pair-gather layout: idx16 = table_row//2 wrapped per block, mask weights
    pair_idx = (idx_arr // 2).astype(np.int16)          # [NCORES, P, S]
    par = (idx_arr % 2).astype(np.float32)              # 0 -> even half, 1 -> odd
    wm = np.zeros((NCORES, P, 4 * S), np.float32)
    wm[:, :, 0::4] = w_arr * (1.0 - par)
    wm[:, :, 1::4] = w_arr * (1.0 - par)
    wm[:, :, 2::4] = w_arr * par
    wm[:, :, 3::4] = w_arr * par
    idx16 = np.zeros((NCORES, P, 8 * S), np.int16)
    for j in range(nblocks):
        t = int(tj[j])
        if t == 0:
            continue
        c0 = int(colbase[j])
        for c in range(NCORES):
            flat = pair_idx[c, :, c0 : c0 + t].T.reshape(-1)      # i = s*128+p
            wrapped = flat.reshape(-1, 16).T                       # [16, 8t]
            idx16[c, :, 8 * c0 : 8 * (c0 + t)] = np.tile(wrapped, (8, 1))

    x_sh = np.zeros((NCORES, shard_n, CIN), np.float32)
    for c in range(NCORES):
        ranks = np.arange(c, N, NCORES)
        x_sh[c, : len(ranks)] = x[order[ranks]]

    return dict(
        order=order,
        shard_n=shard_n,
        nblocks=nblocks,
        tj=tj.astype(int),
        colbase=colbase.astype(int),
        S=S,
        idx_arr=idx_arr,
        w_arr=w_arr,
        wm=wm,
        idx16=idx16,
        x_sh=x_sh,
    )


def _build_program(nblocks, tj, colbase, S, shard_n):
    nc = bacc.Bacc()

    x_ext = nc.declare_dram_parameter("x", [shard_n, CIN], F32, isOutput=False)
    idx_ext = nc.declare_dram_parameter("idx", [P, 8 * S], mybir.dt.int16, isOutput=False)
    wsm_ext = nc.declare_dram_parameter("wsm", [P, S], F32, isOutput=False)
    wpr_ext = nc.declare_dram_parameter("wpr", [P, 4 * S], BF16, isOutput=False)
    w1_ext = nc.declare_dram_parameter("w1", [CIN, CH], F32, isOutput=False)
    cw0_ext = nc.declare_dram_parameter("cw0", [CH, CH], F32, isOutput=False)
    cw1_ext = nc.declare_dram_parameter("cw1", [CH, CH], F32, isOutput=False)
    lin1_ext = nc.declare_dram_parameter("lin1", [CH, CH], F32, isOutput=False)
    lin2_ext = nc.declare_dram_parameter("lin2", [CH, COUT], F32, isOutput=False)
    names = []
    for ell in range(3):
        names += [f"bn{ell}_g", f"bn{ell}_b", f"bn{ell}_m", f"bn{ell}_v", f"cb{ell}"]
    names += ["l1b"]
    vec_exts = {
        nm: nc.declare_dram_parameter(nm, [P, CH], F32, isOutput=False) for nm in names
    }
    l2b_ext = nc.declare_dram_parameter("l2b", [P, COUT], F32, isOutput=False)
    y_ext = nc.declare_dram_parameter("y", [shard_n, COUT], F32, isOutput=True)

    TJMAX = int(max(tj))

    with TileContext(nc) as tc:
        with (
            tc.tile_pool(name="const", bufs=1) as constp,
            tc.tile_pool(name="gpool", bufs=3) as gp,
            tc.tile_pool(name="work", bufs=3) as wk,
            tc.tile_pool(name="psum", bufs=2, space="PSUM") as pp,
            tc.tile_pool(name="psum2", bufs=2, space="PSUM") as pp2,
            tc.tile_pool(name="dram", bufs=1, space="DRAM") as dp,
        ):
            # ---- persistent SBUF ----
            idx_all = constp.tile([P, 8 * S], mybir.dt.int16)
            nc.sync.dma_start(out=idx_all[:], in_=idx_ext[:])
            wsm_all = constp.tile([P, S], F32)
            nc.sync.dma_start(out=wsm_all[:], in_=wsm_ext[:])
            wpr_all = constp.tile([P, 4 * S], BF16)
            nc.sync.dma_start(out=wpr_all[:], in_=wpr_ext[:])
            ident = constp.tile([P, P], F32)
            make_identity(nc, ident[:])

            tloc = constp.tile([P, nblocks * CH], F32)
            h = constp.tile([P, nblocks * CH], F32)
            for j in range(nblocks):
                nc.sync.dma_start(
                    out=h[:, j * CH : (j + 1) * CH],
                    in_=x_ext[j * P : (j + 1) * P, :],
                )

            Wt = {}
            for nm, ext in (
                ("w1", w1_ext),
                ("cw0", cw0_ext),
                ("cw1", cw1_ext),
                ("lin1", lin1_ext),
            ):
                t = constp.tile([P, CH], F32, name=f"W_{nm}")
                nc.sync.dma_start(out=t[:], in_=ext[:])
                Wt[nm] = t
            lin2_t = constp.tile([P, COUT], F32)
            nc.sync.dma_start(out=lin2_t[:], in_=lin2_ext[:])
            vec_t = {}
            for nm, ext in vec_exts.items():
                t = constp.tile([P, CH], F32, name=f"v_{nm}")
                nc.sync.dma_start(out=t[:], in_=ext[:])
                vec_t[nm] = t
            l2b_t = constp.tile([P, COUT], F32)
            nc.sync.dma_start(out=l2b_t[:], in_=l2b_ext[:])

            # ---- fold BN into weights/bias ----
            Wp = {}
            biasp = {}
            for ell, wname in ((0, "w1"), (1, "cw0"), (2, "cw1")):
                g = vec_t[f"bn{ell}_g"]
                b = vec_t[f"bn{ell}_b"]
                m = vec_t[f"bn{ell}_m"]
                v = vec_t[f"bn{ell}_v"]
                cb = vec_t[f"cb{ell}"]
                s_t = constp.tile([P, CH], F32, name=f"s{ell}")
                tmp = wk.tile([P, CH], F32, tag="fold")
                nc.vector.tensor_scalar_add(out=tmp[:], in0=v[:], scalar1=EPS)
                nc.scalar.activation(
                    out=tmp[:], in_=tmp[:], func=mybir.ActivationFunctionType.Sqrt
                )
                nc.vector.reciprocal(out=s_t[:], in_=tmp[:])
                nc.vector.tensor_mul(out=s_t[:], in0=s_t[:], in1=g[:])
                wp = constp.tile([P, CH], F32, name=f"Wp{ell}")
                nc.vector.tensor_mul(out=wp[:], in0=Wt[wname][:], in1=s_t[:])
                Wp[ell] = wp
                bp = constp.tile([P, CH], F32, name=f"bias{ell}")
                tmp2 = wk.tile([P, CH], F32, tag="fold")
                nc.vector.tensor_mul(out=tmp2[:], in0=m[:], in1=s_t[:])
                nc.vector.tensor_sub(out=bp[:], in0=b[:], in1=tmp2[:])
                tmp3 = wk.tile([P, CH], F32, tag="fold")
                nc.vector.tensor_mul(out=tmp3[:], in0=cb[:], in1=s_t[:])
                nc.vector.tensor_add(out=bp[:], in0=bp[:], in1=tmp3[:])
                biasp[ell] = bp

            # ---- degree / dis ----
            dis = constp.tile([P, nblocks], F32)
            for j in range(nblocks):
                t = int(tj[j])
                c0 = int(colbase[j])
                dsum = wk.tile([P, 1], F32, tag="dsum")
                if t > 0:
                    nc.vector.reduce_sum(
                        out=dsum[:],
                        in_=wsm_all[:, c0 : c0 + t],
                        axis=mybir.AxisListType.X,
                    )
                    nc.vector.tensor_scalar_add(out=dsum[:], in0=dsum[:], scalar1=1.0)
                else:
                    nc.vector.memset(dsum[:], 1.0)
                nc.scalar.activation(
                    out=dsum[:],
                    in_=dsum[:],
                    func=mybir.ActivationFunctionType.Sqrt,
                )
                nc.vector.reciprocal(out=dis[:, j : j + 1], in_=dsum[:])

            # ---- DRAM intermediates ----
            tables = []
            ag_ins = []
            for ell in range(3):
                ag_in = dp.tile([shard_n, CH], BF16, name=f"agin{ell}")
                table = dp.tile(
                    [NCORES * shard_n, CH], BF16, name=f"table{ell}", addr_space="Shared"
                )
                ag_ins.append(ag_in)
                tables.append(table)

            lrelu = mybir.ActivationFunctionType.Lrelu

            def emit_A(ell, j):
                # stage dis*h directly; h@W is applied after aggregation
                tsl = tloc[:, j * CH : (j + 1) * CH]
                nc.vector.tensor_scalar_mul(
                    out=tsl, in0=h[:, j * CH : (j + 1) * CH],
                    scalar1=dis[:, j : j + 1],
                )
                stg = wk.tile([P, CH], BF16, tag="stg")
                nc.scalar.activation(
                    out=stg[:], in_=tsl, func=mybir.ActivationFunctionType.Copy
                )
                nc.sync.dma_start(out=ag_ins[ell][j * P : (j + 1) * P, :], in_=stg[:])

            def emit_AG(ell):
                nc.gpsimd.collective_compute(
                    "AllGather",
                    mybir.AluOpType.bypass,
                    replica_groups=[list(range(NCORES))],
                    ins=[ag_ins[ell][:]],
                    outs=[tables[ell][:]],
                )

            def emit_CD(ell, j):
                t = int(tj[j])
                c0 = int(colbase[j])
                acc = wk.tile([P, CH], F32, tag="acc")
                if t > 0:
                    g_t = gp.tile([P, TJMAX * 2 * CH], BF16, tag="g")
                    nc.gpsimd.dma_gather(
                        g_t[:, : t * 2 * CH].rearrange(
                            "p (t c) -> p t c", c=2 * CH
                        ),
                        tables[ell][:].rearrange("(r two) c -> r (two c)", two=2),
                        idx_all[:, 8 * c0 : 8 * (c0 + t)],
                        128 * t,
                        128 * t,
                        2 * CH,
                        elem_step=2 * CH,
                        single_packet=False,
                    )
                    # weighted multiply with parity-mask weights (2x bf16 mode)
                    gv = g_t[:, : t * 2 * CH].rearrange(
                        "p (tn h two) -> p tn h two", h=CH // 2, two=2
                    )
                    wv = (
                        wpr_all[:, 4 * c0 : 4 * (c0 + t)]
                        .rearrange("p (tn two) -> p tn two", two=2)
                        .unsqueeze(2)
                        .to_broadcast((P, 2 * t, CH // 2, 2))
                    )
                    nc.vector.tensor_tensor(
                        out=gv, in0=gv, in1=wv, op=mybir.AluOpType.mult
                    )
                    # tree-fold over 2t half-rows (contiguous bf16 adds)
                    n = 2 * t
                    while n > 2:
                        hh = n // 2
                        nc.vector.tensor_tensor(
                            out=g_t[:, : hh * CH],
                            in0=g_t[:, : hh * CH],
                            in1=g_t[:, (n - hh) * CH : n * CH],
                            op=mybir.AluOpType.add,
                        )
                        n = n - hh
                    if n == 2:
                        nc.vector.tensor_tensor(
                            out=acc[:],
                            in0=g_t[:, :CH],
                            in1=g_t[:, CH : 2 * CH],
                            op=mybir.AluOpType.add,
                        )
                    else:
                        nc.vector.tensor_copy(out=acc[:], in_=g_t[:, :CH])
                    nc.vector.tensor_add(
                        out=acc[:], in0=acc[:], in1=tloc[:, j * CH : (j + 1) * CH]
                    )
                else:
                    nc.vector.tensor_copy(
                        out=acc[:], in_=tloc[:, j * CH : (j + 1) * CH]
                    )
                # facc = acc*dis; z = facc @ Wp; acc2 = z + bias (+h); lrelu
                nc.vector.tensor_scalar_mul(
                    out=acc[:], in0=acc[:], scalar1=dis[:, j : j + 1]
                )
                htp = pp.tile([P, P], F32, tag="htp")
                nc.tensor.transpose(out=htp[:], in_=acc[:], identity=ident[:])
                hts = wk.tile([P, P], F32, tag="hts")
                nc.vector.tensor_copy(out=hts[:], in_=htp[:])
                zp = pp2.tile([P, CH], F32, tag="zp")
                nc.tensor.matmul(
                    out=zp[:], lhsT=hts[:], rhs=Wp[ell][:], start=True, stop=True
                )
                acc2 = wk.tile([P, CH], F32, tag="acc2")
                nc.vector.tensor_add(out=acc2[:], in0=zp[:], in1=biasp[ell][:])
                if ell >= 1:
                    nc.vector.tensor_add(
                        out=acc2[:], in0=acc2[:], in1=h[:, j * CH : (j + 1) * CH]
                    )
                nc.scalar.activation(
                    out=h[:, j * CH : (j + 1) * CH], in_=acc2[:], func=lrelu, alpha=0.01
                )

            def emit_head(j):
                hsl = h[:, j * CH : (j + 1) * CH]
                htp = pp.tile([P, P], F32, tag="htp")
                nc.tensor.transpose(out=htp[:], in_=hsl, identity=ident[:])
                hts = wk.tile([P, P], F32, tag="hts")
                nc.vector.tensor_copy(out=hts[:], in_=htp[:])
                z1p = pp2.tile([P, CH], F32, tag="zp")
                nc.tensor.matmul(
                    out=z1p[:], lhsT=hts[:], rhs=Wt["lin1"][:], start=True, stop=True
                )
                z1 = wk.tile([P, CH], F32, tag="z1")
                nc.vector.tensor_add(out=z1[:], in0=z1p[:], in1=vec_t["l1b"][:])
                nc.scalar.activation(out=z1[:], in_=z1[:], func=lrelu, alpha=0.01)
                z1tp = pp.tile([P, P], F32, tag="htp")
                nc.tensor.transpose(out=z1tp[:], in_=z1[:], identity=ident[:])
                z1ts = wk.tile([P, P], F32, tag="hts")
                nc.vector.tensor_copy(out=z1ts[:], in_=z1tp[:])
                z2p = pp2.tile([P, COUT], F32, tag="z2p")
                nc.tensor.matmul(
                    out=z2p[:], lhsT=z1ts[:], rhs=lin2_t[:], start=True, stop=True
                )
                yt = wk.tile([P, COUT], F32, tag="yt")
                nc.vector.tensor_add(out=yt[:], in0=z2p[:], in1=l2b_t[:])
                nc.sync.dma_start(out=y_ext[j * P : (j + 1) * P, :], in_=yt[:])

            # layer 0 table from x, then pipeline: CD(l) interleaved with A(l+1)
            for j in range(nblocks):
                emit_A(0, j)
            emit_AG(0)
            for ell in range(3):
                for j in range(nblocks):
                    emit_CD(ell, j)
                    if ell < 2:
                        emit_A(ell + 1, j)
                    else:
                        emit_head(j)
                if ell < 2:
                    emit_AG(ell + 1)

    nc.compile()
    return nc


def kernel(**inputs):
    x = np.asarray(inputs["x"], np.float32)
    edge_index = np.asarray(inputs["edge_index"], np.int64)
    edge_weights = np.asarray(inputs["edge_weights"], np.float32)

    prep = _host_prep(x, edge_index, edge_weights)
    nblocks, tj, colbase, S, shard_n = (
        prep["nblocks"],
        prep["tj"],
        prep["colbase"],
        prep["S"],
        prep["shard_n"],
    )

    nc = _build_program(nblocks, tj, colbase, S, shard_n)

    rep = lambda v: np.tile(np.asarray(v, np.float32)[None, :], (P, 1))
    in_maps = []
    for c in range(NCORES):
        m = {
            "x": prep["x_sh"][c],
            "idx": prep["idx16"][c],
            "wsm": prep["w_arr"][c],
            "wpr": np.ascontiguousarray(prep["wm"][c].astype(ml_dtypes.bfloat16)),
            "w1": np.asarray(inputs["w1"], np.float32),
            "cw0": np.asarray(inputs["conv_ws"], np.float32)[0],
            "cw1": np.asarray(inputs["conv_ws"], np.float32)[1],
            "lin1": np.asarray(inputs["lin1_w"], np.float32),
            "lin2": np.asarray(inputs["lin2_w"], np.float32),
            "l1b": rep(inputs["lin1_b"]),
            "l2b": rep(inputs["lin2_b"]),
        }
        for ell in range(3):
            if ell == 0:
                g, b, mm, v = (
                    inputs["bn1_g"],
                    inputs["bn1_b"],
                    inputs["bn1_m"],
                    inputs["bn1_v"],
                )
                cb = inputs["b1"]
            else:
                g, b, mm, v = (
                    np.asarray(inputs["bns_g"])[ell - 1],
                    np.asarray(inputs["bns_b"])[ell - 1],
                    np.asarray(inputs["bns_m"])[ell - 1],
                    np.asarray(inputs["bns_v"])[ell - 1],
                )
                cb = np.asarray(inputs["conv_bs"])[ell - 1]
            m[f"bn{ell}_g"] = rep(g)
            m[f"bn{ell}_b"] = rep(b)
            m[f"bn{ell}_m"] = rep(mm)
            m[f"bn{ell}_v"] = rep(v)
            m[f"cb{ell}"] = rep(cb)
        in_maps.append(m)

    res = run_bass_kernel_spmd(nc, in_maps, core_ids=list(range(NCORES)))
    global _last_results
    _last_results = res

    out = np.empty((N, COUT), np.float32)
    order = prep["order"]
    for c in range(NCORES):
        ranks = np.arange(c, N, NCORES)
        out[order[ranks]] = res.results[c]["y"][: len(ranks)]
    return out


# revision 8
# speedup vs baseline: 1.0170x; 1.0170x over previous
"""GCN message-passing kernel for 8 trn2 NeuronCores (bass/Tile).

Sharding: nodes are degree-sorted and dealt round-robin across 8 cores
(graph-parallel, dst-sharded).  Each core computes h@W for its own node
shard, chunked AllGathers replicate the scaled table, and each core then
aggregates its own destination nodes with per-edge indirect-DMA gathers
followed by an on-chip weighted tree-fold reduction on DVE (weights are
broadcast from SBUF via paired bf16 APs; no per-edge weight stream from
HBM).  Small 128x128 weights are replicated; BN (eval mode) is folded
into the weights/bias on device.
"""

import sys

sys.path.insert(0, "/opt/trn_rl_repo")

import numpy as np
import ml_dtypes

import concourse.bass as bass
import concourse.bacc as bacc
import concourse.mybir as mybir
from concourse.bass_utils import run_bass_kernel_spmd
from concourse.masks import make_identity
from concourse.tile import TileContext

N = 50000
E = 800000
CIN = 128
CH = 128
COUT = 64
EPS = 1e-5
NCORES = 8
P = 128

F32 = mybir.dt.float32
BF16 = mybir.dt.bfloat16
I32 = mybir.dt.int32

SHARD_N = 6272          # 49 blocks * 128
NBLOCKS = SHARD_N // P  # 49
# AllGather chunk boundaries, in blocks
CHUNK_BLOCKS = [(0, 13), (13, 25), (25, 37), (37, 49)]


def _host_prep(x, edge_index, edge_weights):
    """Pure index/layout work: shard nodes, build per-core slot layout."""
    src = edge_index[0].astype(np.int64)
    dst = edge_index[1].astype(np.int64)
    ew = edge_weights.astype(np.float32)

    deg = np.bincount(dst, minlength=N)

    order = np.argsort(deg, kind="stable")  # node_of_rank
    rank_of_node = np.empty(N, np.int64)
    rank_of_node[order] = np.arange(N)

    shard_n = SHARD_N
    nblocks = NBLOCKS
    tj = np.zeros(nblocks, np.int64)
    degs_by_rank = deg[order]
    for j in range(nblocks):
        lo, hi = j * 1024, min((j + 1) * 1024, N)
        tj[j] = degs_by_rank[lo:hi].max() if lo < N else 1
    colbase = np.concatenate([[0], np.cumsum(tj)])
    S = int(colbase[-1])

    # table row of node n (AllGather layout: [core0 shard | core1 shard | ...])
    r = rank_of_node
    table_row = (r % NCORES) * shard_n + (r // NCORES)

    idx_arr = np.zeros((NCORES, P, S), np.int32)
    w_arr = np.zeros((NCORES, P, S), np.float32)

    dr = rank_of_node[dst]
    e_order = np.argsort(dr, kind="stable")
    dr_s = dr[e_order]
    src_s = src[e_order]
    ew_s = ew[e_order]
    starts = np.searchsorted(dr_s, np.arange(N))
    slot = np.arange(len(dr_s)) - starts[dr_s]

    core = dr_s % NCORES
    local = dr_s // NCORES
    block = local // P
    part = local % P
    col = colbase[block] + slot
    idx_arr[core, part, col] = table_row[src_s].astype(np.int32)
    w_arr[core, part, col] = ew_s

    # pair-gather layout: idx16 = table_row//2 wrapped per block, mask weights
    pair_idx = (idx_arr // 2).astype(np.int16)          # [NCORES, P, S]
    par = (idx_arr % 2).astype(np.float32)              # 0 -> even half, 1 -> odd
    wm = np.zeros((NCORES, P, 4 * S), np.float32)
    wm[:, :, 0::4] = w_arr * (1.0 - par)
    wm[:, :, 1::4] = w_arr * (1.0 - par)
    wm[:, :, 2::4] = w_arr * par
    wm[:, :, 3::4] = w_arr * par
    idx16 = np.zeros((NCORES, P, 8 * S), np.int16)
    for j in range(nblocks):
        t = int(tj[j])
        if t == 0:
            continue
        c0 = int(colbase[j])
        for c in range(NCORES):
            flat = pair_idx[c, :, c0 : c0 + t].T.reshape(-1)      # i = s*128+p
            wrapped = flat.reshape(-1, 16).T                       # [16, 8t]
            idx16[c, :, 8 * c0 : 8 * (c0 + t)] = np.tile(wrapped, (8, 1))

    x_sh = np.zeros((NCORES, shard_n, CIN), np.float32)
    for c in range(NCORES):
        ranks = np.arange(c, N, NCORES)
        x_sh[c, : len(ranks)] = x[order[ranks]]

    return dict(
        order=order,
        shard_n=shard_n,
        nblocks=nblocks,
        tj=tj.astype(int),
        colbase=colbase.astype(int),
        S=S,
        idx_arr=idx_arr,
        w_arr=w_arr,
        wm=wm,
        idx16=idx16,
        x_sh=x_sh,
    )


def _build_program(nblocks, tj, colbase, S, shard_n):
    nc = bacc.Bacc()

    x_ext = nc.declare_dram_parameter("x", [shard_n, CIN], F32, isOutput=False)
    idx_ext = nc.declare_dram_parameter("idx", [P, 8 * S], mybir.dt.int16, isOutput=False)
    wsm_ext = nc.declare_dram_parameter("wsm", [P, S], F32, isOutput=False)
    wpr_ext = nc.declare_dram_parameter("wpr", [P, 4 * S], BF16, isOutput=False)
    w1_ext = nc.declare_dram_parameter("w1", [CIN, CH], F32, isOutput=False)
    cw0_ext = nc.declare_dram_parameter("cw0", [CH, CH], F32, isOutput=False)
    cw1_ext = nc.declare_dram_parameter("cw1", [CH, CH], F32, isOutput=False)
    lin1_ext = nc.declare_dram_parameter("lin1", [CH, CH], F32, isOutput=False)
    lin2_ext = nc.declare_dram_parameter("lin2", [CH, COUT], F32, isOutput=False)
    names = []
    for ell in range(3):
        names += [f"bn{ell}_g", f"bn{ell}_b", f"bn{ell}_m", f"bn{ell}_v", f"cb{ell}"]
    names += ["l1b"]
    vec_exts = {
        nm: nc.declare_dram_parameter(nm, [P, CH], F32, isOutput=False) for nm in names
    }
    l2b_ext = nc.declare_dram_parameter("l2b", [P, COUT], F32, isOutput=False)
    y_ext = nc.declare_dram_parameter("y", [shard_n, COUT], F32, isOutput=True)

    TJMAX = int(max(tj))

    with TileContext(nc) as tc:
        with (
            tc.tile_pool(name="const", bufs=1) as constp,
            tc.tile_pool(name="gpool", bufs=4) as gp,
            tc.tile_pool(name="work", bufs=3) as wk,
            tc.tile_pool(name="psum", bufs=2, space="PSUM") as pp,
            tc.tile_pool(name="psum2", bufs=2, space="PSUM") as pp2,
            tc.tile_pool(name="dram", bufs=1, space="DRAM") as dp,
        ):
            # ---- persistent SBUF ----
            idx_all = constp.tile([P, 8 * S], mybir.dt.int16)
            nc.sync.dma_start(out=idx_all[:], in_=idx_ext[:])
            wsm_all = constp.tile([P, S], F32)
            nc.sync.dma_start(out=wsm_all[:], in_=wsm_ext[:])
            wpr_all = constp.tile([P, 4 * S], BF16)
            nc.sync.dma_start(out=wpr_all[:], in_=wpr_ext[:])
            ident = constp.tile([P, P], F32)
            make_identity(nc, ident[:])

            tloc = constp.tile([P, nblocks * CH], F32)
            h = constp.tile([P, nblocks * CH], F32)
            for j in range(nblocks):
                nc.sync.dma_start(
                    out=h[:, j * CH : (j + 1) * CH],
                    in_=x_ext[j * P : (j + 1) * P, :],
                )

            Wt = {}
            for nm, ext in (
                ("w1", w1_ext),
                ("cw0", cw0_ext),
                ("cw1", cw1_ext),
                ("lin1", lin1_ext),
            ):
                t = constp.tile([P, CH], F32, name=f"W_{nm}")
                nc.sync.dma_start(out=t[:], in_=ext[:])
                Wt[nm] = t
            lin2_t = constp.tile([P, COUT], F32)
            nc.sync.dma_start(out=lin2_t[:], in_=lin2_ext[:])
            vec_t = {}
            for nm, ext in vec_exts.items():
                t = constp.tile([P, CH], F32, name=f"v_{nm}")
                nc.sync.dma_start(out=t[:], in_=ext[:])
                vec_t[nm] = t
            l2b_t = constp.tile([P, COUT], F32)
            nc.sync.dma_start(out=l2b_t[:], in_=l2b_ext[:])

            # ---- fold BN into weights/bias ----
            Wp = {}
            biasp = {}
            for ell, wname in ((0, "w1"), (1, "cw0"), (2, "cw1")):
                g = vec_t[f"bn{ell}_g"]
                b = vec_t[f"bn{ell}_b"]
                m = vec_t[f"bn{ell}_m"]
                v = vec_t[f"bn{ell}_v"]
                cb = vec_t[f"cb{ell}"]
                s_t = constp.tile([P, CH], F32, name=f"s{ell}")
                tmp = wk.tile([P, CH], F32, tag="fold")
                nc.vector.tensor_scalar_add(out=tmp[:], in0=v[:], scalar1=EPS)
                nc.scalar.activation(
                    out=tmp[:], in_=tmp[:], func=mybir.ActivationFunctionType.Sqrt
                )
                nc.vector.reciprocal(out=s_t[:], in_=tmp[:])
                nc.vector.tensor_mul(out=s_t[:], in0=s_t[:], in1=g[:])
                wp = constp.tile([P, CH], F32, name=f"Wp{ell}")
                nc.vector.tensor_mul(out=wp[:], in0=Wt[wname][:], in1=s_t[:])
                Wp[ell] = wp
                bp = constp.tile([P, CH], F32, name=f"bias{ell}")
                tmp2 = wk.tile([P, CH], F32, tag="fold")
                nc.vector.tensor_mul(out=tmp2[:], in0=m[:], in1=s_t[:])
                nc.vector.tensor_sub(out=bp[:], in0=b[:], in1=tmp2[:])
                tmp3 = wk.tile([P, CH], F32, tag="fold")
                nc.vector.tensor_mul(out=tmp3[:], in0=cb[:], in1=s_t[:])
                nc.vector.tensor_add(out=bp[:], in0=bp[:], in1=tmp3[:])
                biasp[ell] = bp

            # ---- degree / dis ----
            dis = constp.tile([P, nblocks], F32)
            for j in range(nblocks):
                t = int(tj[j])
                c0 = int(colbase[j])
                dsum = wk.tile([P, 1], F32, tag="dsum")
                if t > 0:
                    nc.vector.reduce_sum(
                        out=dsum[:],
                        in_=wsm_all[:, c0 : c0 + t],
                        axis=mybir.AxisListType.X,
                    )
                    nc.vector.tensor_scalar_add(out=dsum[:], in0=dsum[:], scalar1=1.0)
                else:
                    nc.vector.memset(dsum[:], 1.0)
                nc.scalar.activation(
                    out=dsum[:],
                    in_=dsum[:],
                    func=mybir.ActivationFunctionType.Sqrt,
                )
                nc.vector.reciprocal(out=dis[:, j : j + 1], in_=dsum[:])

            # ---- DRAM intermediates ----
            tables = []
            ag_ins = []
            for ell in range(3):
                ag_in = dp.tile([shard_n, CH], BF16, name=f"agin{ell}")
                table = dp.tile(
                    [NCORES * shard_n, CH], BF16, name=f"table{ell}", addr_space="Shared"
                )
                ag_ins.append(ag_in)
                tables.append(table)

            lrelu = mybir.ActivationFunctionType.Lrelu

            def emit_A(ell, j):
                hsl = h[:, j * CH : (j + 1) * CH]
                htp = pp.tile([P, P], F32, tag="htp")
                nc.tensor.transpose(out=htp[:], in_=hsl, identity=ident[:])
                hts = wk.tile([P, P], F32, tag="hts")
                nc.vector.tensor_copy(out=hts[:], in_=htp[:])
                zp = pp2.tile([P, CH], F32, tag="zp")
                nc.tensor.matmul(
                    out=zp[:], lhsT=hts[:], rhs=Wp[ell][:], start=True, stop=True
                )
                tsl = tloc[:, j * CH : (j + 1) * CH]
                nc.vector.tensor_scalar_mul(
                    out=tsl, in0=zp[:], scalar1=dis[:, j : j + 1]
                )
                stg = wk.tile([P, CH], BF16, tag="stg")
                nc.scalar.activation(
                    out=stg[:], in_=tsl, func=mybir.ActivationFunctionType.Copy
                )
                nc.sync.dma_start(out=ag_ins[ell][j * P : (j + 1) * P, :], in_=stg[:])

            def emit_AG(ell):
                nc.gpsimd.collective_compute(
                    "AllGather",
                    mybir.AluOpType.bypass,
                    replica_groups=[list(range(NCORES))],
                    ins=[ag_ins[ell][:]],
                    outs=[tables[ell][:]],
                )

            def emit_CD(ell, j):
                t = int(tj[j])
                c0 = int(colbase[j])
                acc = wk.tile([P, CH], F32, tag="acc")
                if t > 0:
                    g_t = gp.tile([P, TJMAX * 2 * CH], BF16, tag="g")
                    nc.gpsimd.dma_gather(
                        g_t[:, : t * 2 * CH].rearrange(
                            "p (t c) -> p t c", c=2 * CH
                        ),
                        tables[ell][:].rearrange("(r two) c -> r (two c)", two=2),
                        idx_all[:, 8 * c0 : 8 * (c0 + t)],
                        128 * t,
                        128 * t,
                        2 * CH,
                        elem_step=2 * CH,
                        single_packet=False,
                    )
                    # weighted multiply with parity-mask weights (2x bf16 mode)
                    gv = g_t[:, : t * 2 * CH].rearrange(
                        "p (tn h two) -> p tn h two", h=CH // 2, two=2
                    )
                    wv = (
                        wpr_all[:, 4 * c0 : 4 * (c0 + t)]
                        .rearrange("p (tn two) -> p tn two", two=2)
                        .unsqueeze(2)
                        .to_broadcast((P, 2 * t, CH // 2, 2))
                    )
                    nc.vector.tensor_tensor(
                        out=gv, in0=gv, in1=wv, op=mybir.AluOpType.mult
                    )
                    # tree-fold over 2t half-rows (contiguous bf16 adds)
                    n = 2 * t
                    while n > 2:
                        hh = n // 2
                        nc.vector.tensor_tensor(
                            out=g_t[:, : hh * CH],
                            in0=g_t[:, : hh * CH],
                            in1=g_t[:, (n - hh) * CH : n * CH],
                            op=mybir.AluOpType.add,
                        )
                        n = n - hh
                    if n == 2:
                        nc.vector.tensor_tensor(
                            out=acc[:],
                            in0=g_t[:, :CH],
                            in1=g_t[:, CH : 2 * CH],
                            op=mybir.AluOpType.add,
                        )
                    else:
                        nc.vector.tensor_copy(out=acc[:], in_=g_t[:, :CH])
                    nc.vector.tensor_add(
                        out=acc[:], in0=acc[:], in1=tloc[:, j * CH : (j + 1) * CH]
                    )
                else:
                    nc.vector.tensor_copy(
                        out=acc[:], in_=tloc[:, j * CH : (j + 1) * CH]
                    )
                # acc = acc*dis + bias   (fused)
                nc.vector.scalar_tensor_tensor(
                    out=acc[:],
                    in0=acc[:],
                    scalar=dis[:, j : j + 1],
                    in1=biasp[ell][:],
                    op0=mybir.AluOpType.mult,
                    op1=mybir.AluOpType.add,
                )
                if ell >= 1:
                    nc.vector.tensor_add(
                        out=acc[:], in0=acc[:], in1=h[:, j * CH : (j + 1) * CH]
                    )
                nc.scalar.activation(
                    out=h[:, j * CH : (j + 1) * CH], in_=acc[:], func=lrelu, alpha=0.01
                )

            def emit_head(j):
                hsl = h[:, j * CH : (j + 1) * CH]
                htp = pp.tile([P, P], F32, tag="htp")
                nc.tensor.transpose(out=htp[:], in_=hsl, identity=ident[:])
                hts = wk.tile([P, P], F32, tag="hts")
                nc.vector.tensor_copy(out=hts[:], in_=htp[:])
                z1p = pp2.tile([P, CH], F32, tag="zp")
                nc.tensor.matmul(
                    out=z1p[:], lhsT=hts[:], rhs=Wt["lin1"][:], start=True, stop=True
                )
                z1 = wk.tile([P, CH], F32, tag="z1")
                nc.vector.tensor_add(out=z1[:], in0=z1p[:], in1=vec_t["l1b"][:])
                nc.scalar.activation(out=z1[:], in_=z1[:], func=lrelu, alpha=0.01)
                z1tp = pp.tile([P, P], F32, tag="htp")
                nc.tensor.transpose(out=z1tp[:], in_=z1[:], identity=ident[:])
                z1ts = wk.tile([P, P], F32, tag="hts")
                nc.vector.tensor_copy(out=z1ts[:], in_=z1tp[:])
                z2p = pp2.tile([P, COUT], F32, tag="z2p")
                nc.tensor.matmul(
                    out=z2p[:], lhsT=z1ts[:], rhs=lin2_t[:], start=True, stop=True
                )
                yt = wk.tile([P, COUT], F32, tag="yt")
                nc.vector.tensor_add(out=yt[:], in0=z2p[:], in1=l2b_t[:])
                nc.sync.dma_start(out=y_ext[j * P : (j + 1) * P, :], in_=yt[:])

            # layer 0 table from x, then pipeline: CD(l) interleaved with A(l+1)
            for j in range(nblocks):
                emit_A(0, j)
            emit_AG(0)
            for ell in range(3):
                for j in range(nblocks):
                    emit_CD(ell, j)
                    if ell < 2:
                        emit_A(ell + 1, j)
                    else:
                        emit_head(j)
                if ell < 2:
                    emit_AG(ell + 1)

    nc.compile()
    return nc


def kernel(**inputs):
    x = np.asarray(inputs["x"], np.float32)
    edge_index = np.asarray(inputs["edge_index"], np.int64)
    edge_weights = np.asarray(inputs["edge_weights"], np.float32)

    prep = _host_prep(x, edge_index, edge_weights)
    nblocks, tj, colbase, S, shard_n = (
        prep["nblocks"],
        prep["tj"],
        prep["colbase"],
        prep["S"],
        prep["shard_n"],
    )

    nc = _build_program(nblocks, tj, colbase, S, shard_n)

    rep = lambda v: np.tile(np.asarray(v, np.float32)[None, :], (P, 1))
    in_maps = []
    for c in range(NCORES):
        m = {
            "x": prep["x_sh"][c],
            "idx": prep["idx16"][c],
            "wsm": prep["w_arr"][c],
            "wpr": np.ascontiguousarray(prep["wm"][c].astype(ml_dtypes.bfloat16)),
            "w1": np.asarray(inputs["w1"], np.float32),
            "cw0": np.asarray(inputs["conv_ws"], np.float32)[0],
            "cw1": np.asarray(inputs["conv_ws"], np.float32)[1],
            "lin1": np.asarray(inputs["lin1_w"], np.float32),
            "lin2": np.asarray(inputs["lin2_w"], np.float32),
            "l1b": rep(inputs["lin1_b"]),
            "l2b": rep(inputs["lin2_b"]),
        }
        for ell in range(3):
            if ell == 0:
                g, b, mm, v = (
                    inputs["bn1_g"],
                    inputs["bn1_b"],
                    inputs["bn1_m"],
                    inputs["bn1_v"],
                )
                cb = inputs["b1"]
            else:
                g, b, mm, v = (
                    np.asarray(inputs["bns_g"])[ell - 1],
                    np.asarray(inputs["bns_b"])[ell - 1],
                    np.asarray(inputs["bns_m"])[ell - 1],
                    np.asarray(inputs["bns_v"])[ell - 1],
                )
                cb = np.asarray(inputs["conv_bs"])[ell - 1]
            m[f"bn{ell}_g"] = rep(g)
            m[f"bn{ell}_b"] = rep(b)
            m[f"bn{ell}_m"] = rep(mm)
            m[f"bn{ell}_v"] = rep(v)
            m[f"cb{ell}"] = rep(cb)
        in_maps.append(m)

    res = run_bass_kernel_spmd(nc, in_maps, core_ids=list(range(NCORES)))
    global _last_results
    _last_results = res

    out = np.empty((N, COUT), np.float32)
    order = prep["order"]
    for c in range(NCORES):
        ranks = np.arange(c, N, NCORES)
        out[order[ranks]] = res.results[c]["y"][: len(ranks)]
    return out


# revision 9
# speedup vs baseline: 1.0696x; 1.0517x over previous
"""GCN message-passing kernel for 8 trn2 NeuronCores (bass/Tile).

Sharding: nodes are degree-sorted and dealt round-robin across 8 cores
(graph-parallel, dst-sharded).  Each core computes h@W for its own node
shard, chunked AllGathers replicate the scaled table, and each core then
aggregates its own destination nodes with per-edge indirect-DMA gathers
followed by an on-chip weighted tree-fold reduction on DVE (weights are
broadcast from SBUF via paired bf16 APs; no per-edge weight stream from
HBM).  Small 128x128 weights are replicated; BN (eval mode) is folded
into the weights/bias on device.
"""

import sys

sys.path.insert(0, "/opt/trn_rl_repo")

import numpy as np
import ml_dtypes

import concourse.bass as bass
import concourse.bacc as bacc
import concourse.mybir as mybir
from concourse.bass_utils import run_bass_kernel_spmd
from concourse.masks import make_identity
from concourse.tile import TileContext

N = 50000
E = 800000
CIN = 128
CH = 128
COUT = 64
EPS = 1e-5
NCORES = 8
P = 128

F32 = mybir.dt.float32
BF16 = mybir.dt.bfloat16
I32 = mybir.dt.int32

SHARD_N = 6272          # 49 blocks * 128
NBLOCKS = SHARD_N // P  # 49
# AllGather chunk boundaries, in blocks
CHUNK_BLOCKS = [(0, 13), (13, 25), (25, 37), (37, 49)]


def _host_prep(x, edge_index, edge_weights):
    """Pure index/layout work: shard nodes, build per-core slot layout."""
    src = edge_index[0].astype(np.int64)
    dst = edge_index[1].astype(np.int64)
    ew = edge_weights.astype(np.float32)

    deg = np.bincount(dst, minlength=N)

    order = np.argsort(deg, kind="stable")  # node_of_rank
    rank_of_node = np.empty(N, np.int64)
    rank_of_node[order] = np.arange(N)

    shard_n = SHARD_N
    nblocks = NBLOCKS
    tj = np.zeros(nblocks, np.int64)
    degs_by_rank = deg[order]
    for j in range(nblocks):
        lo, hi = j * 1024, min((j + 1) * 1024, N)
        tj[j] = degs_by_rank[lo:hi].max() if lo < N else 1
    colbase = np.concatenate([[0], np.cumsum(tj)])
    S = int(colbase[-1])

    # table row of node n (AllGather layout: [core0 shard | core1 shard | ...])
    r = rank_of_node
    table_row = (r % NCORES) * shard_n + (r // NCORES)

    idx_arr = np.zeros((NCORES, P, S), np.int32)
    w_arr = np.zeros((NCORES, P, S), np.float32)

    dr = rank_of_node[dst]
    e_order = np.argsort(dr, kind="stable")
    dr_s = dr[e_order]
    src_s = src[e_order]
    ew_s = ew[e_order]
    starts = np.searchsorted(dr_s, np.arange(N))
    slot = np.arange(len(dr_s)) - starts[dr_s]

    core = dr_s % NCORES
    local = dr_s // NCORES
    block = local // P
    part = local % P
    col = colbase[block] + slot
    idx_arr[core, part, col] = table_row[src_s].astype(np.int32)
    w_arr[core, part, col] = ew_s

    # pair-gather layout: idx16 = table_row//2 wrapped per block, mask weights
    pair_idx = (idx_arr // 2).astype(np.int16)          # [NCORES, P, S]
    par = (idx_arr % 2).astype(np.float32)              # 0 -> even half, 1 -> odd
    wm = np.zeros((NCORES, P, 4 * S), np.float32)
    wm[:, :, 0::4] = w_arr * (1.0 - par)
    wm[:, :, 1::4] = w_arr * (1.0 - par)
    wm[:, :, 2::4] = w_arr * par
    wm[:, :, 3::4] = w_arr * par
    idx16 = np.zeros((NCORES, P, 8 * S), np.int16)
    for j in range(nblocks):
        t = int(tj[j])
        if t == 0:
            continue
        c0 = int(colbase[j])
        for c in range(NCORES):
            flat = pair_idx[c, :, c0 : c0 + t].T.reshape(-1)      # i = s*128+p
            wrapped = flat.reshape(-1, 16).T                       # [16, 8t]
            idx16[c, :, 8 * c0 : 8 * (c0 + t)] = np.tile(wrapped, (8, 1))

    x_sh = np.zeros((NCORES, shard_n, CIN), np.float32)
    for c in range(NCORES):
        ranks = np.arange(c, N, NCORES)
        x_sh[c, : len(ranks)] = x[order[ranks]]

    return dict(
        order=order,
        shard_n=shard_n,
        nblocks=nblocks,
        tj=tj.astype(int),
        colbase=colbase.astype(int),
        S=S,
        idx_arr=idx_arr,
        w_arr=w_arr,
        wm=wm,
        idx16=idx16,
        x_sh=x_sh,
    )


def _build_program(nblocks, tj, colbase, S, shard_n):
    nc = bacc.Bacc()

    x_ext = nc.declare_dram_parameter("x", [shard_n, CIN], F32, isOutput=False)
    idx_ext = nc.declare_dram_parameter("idx", [P, 8 * S], mybir.dt.int16, isOutput=False)
    wsm_ext = nc.declare_dram_parameter("wsm", [P, S], F32, isOutput=False)
    wpr_ext = nc.declare_dram_parameter("wpr", [P, 4 * S], BF16, isOutput=False)
    w1_ext = nc.declare_dram_parameter("w1", [CIN, CH], F32, isOutput=False)
    cw0_ext = nc.declare_dram_parameter("cw0", [CH, CH], F32, isOutput=False)
    cw1_ext = nc.declare_dram_parameter("cw1", [CH, CH], F32, isOutput=False)
    lin1_ext = nc.declare_dram_parameter("lin1", [CH, CH], F32, isOutput=False)
    lin2_ext = nc.declare_dram_parameter("lin2", [CH, COUT], F32, isOutput=False)
    names = []
    for ell in range(3):
        names += [f"bn{ell}_g", f"bn{ell}_b", f"bn{ell}_m", f"bn{ell}_v", f"cb{ell}"]
    names += ["l1b"]
    vec_exts = {
        nm: nc.declare_dram_parameter(nm, [P, CH], F32, isOutput=False) for nm in names
    }
    l2b_ext = nc.declare_dram_parameter("l2b", [P, COUT], F32, isOutput=False)
    y_ext = nc.declare_dram_parameter("y", [shard_n, COUT], F32, isOutput=True)

    TJMAX = int(max(tj))

    with TileContext(nc) as tc:
        with (
            tc.tile_pool(name="const", bufs=1) as constp,
            tc.tile_pool(name="gpool", bufs=4) as gp,
            tc.tile_pool(name="work", bufs=3) as wk,
            tc.tile_pool(name="psum", bufs=2, space="PSUM") as pp,
            tc.tile_pool(name="psum2", bufs=2, space="PSUM") as pp2,
            tc.tile_pool(name="dram", bufs=1, space="DRAM") as dp,
        ):
            # ---- persistent SBUF ----
            idx_all = constp.tile([P, 8 * S], mybir.dt.int16)
            nc.sync.dma_start(out=idx_all[:], in_=idx_ext[:])
            wsm_all = constp.tile([P, S], F32)
            nc.sync.dma_start(out=wsm_all[:], in_=wsm_ext[:])
            wpr_all = constp.tile([P, 4 * S], BF16)
            nc.sync.dma_start(out=wpr_all[:], in_=wpr_ext[:])
            ident = constp.tile([P, P], F32)
            make_identity(nc, ident[:])

            tloc = constp.tile([P, nblocks * CH], F32)
            h = constp.tile([P, nblocks * CH], F32)
            for j in range(nblocks):
                eng = nc.sync if j % 2 == 0 else nc.scalar
                eng.dma_start(
                    out=h[:, j * CH : (j + 1) * CH],
                    in_=x_ext[j * P : (j + 1) * P, :],
                )

            Wt = {}
            for nm, ext in (
                ("w1", w1_ext),
                ("cw0", cw0_ext),
                ("cw1", cw1_ext),
                ("lin1", lin1_ext),
            ):
                t = constp.tile([P, CH], F32, name=f"W_{nm}")
                nc.sync.dma_start(out=t[:], in_=ext[:])
                Wt[nm] = t
            lin2_t = constp.tile([P, COUT], F32)
            nc.sync.dma_start(out=lin2_t[:], in_=lin2_ext[:])
            vec_t = {}
            for i_, (nm, ext) in enumerate(vec_exts.items()):
                t = constp.tile([P, CH], F32, name=f"v_{nm}")
                (nc.sync if i_ % 2 == 0 else nc.scalar).dma_start(out=t[:], in_=ext[:])
                vec_t[nm] = t
            l2b_t = constp.tile([P, COUT], F32)
            nc.sync.dma_start(out=l2b_t[:], in_=l2b_ext[:])

            # ---- fold BN into weights/bias ----
            Wp = {}
            biasp = {}
            for ell, wname in ((0, "w1"), (1, "cw0"), (2, "cw1")):
                g = vec_t[f"bn{ell}_g"]
                b = vec_t[f"bn{ell}_b"]
                m = vec_t[f"bn{ell}_m"]
                v = vec_t[f"bn{ell}_v"]
                cb = vec_t[f"cb{ell}"]
                s_t = constp.tile([P, CH], F32, name=f"s{ell}")
                tmp = wk.tile([P, CH], F32, tag="fold")
                nc.vector.tensor_scalar_add(out=tmp[:], in0=v[:], scalar1=EPS)
                nc.scalar.activation(
                    out=tmp[:], in_=tmp[:], func=mybir.ActivationFunctionType.Sqrt
                )
                nc.vector.reciprocal(out=s_t[:], in_=tmp[:])
                nc.vector.tensor_mul(out=s_t[:], in0=s_t[:], in1=g[:])
                wp = constp.tile([P, CH], F32, name=f"Wp{ell}")
                nc.vector.tensor_mul(out=wp[:], in0=Wt[wname][:], in1=s_t[:])
                Wp[ell] = wp
                bp = constp.tile([P, CH], F32, name=f"bias{ell}")
                tmp2 = wk.tile([P, CH], F32, tag="fold")
                nc.vector.tensor_mul(out=tmp2[:], in0=m[:], in1=s_t[:])
                nc.vector.tensor_sub(out=bp[:], in0=b[:], in1=tmp2[:])
                tmp3 = wk.tile([P, CH], F32, tag="fold")
                nc.vector.tensor_mul(out=tmp3[:], in0=cb[:], in1=s_t[:])
                nc.vector.tensor_add(out=bp[:], in0=bp[:], in1=tmp3[:])
                biasp[ell] = bp

            # ---- degree / dis (batched transcendental tail) ----
            dis = constp.tile([P, nblocks], F32)
            draw = constp.tile([P, nblocks], F32, name="draw")
            for j in range(nblocks):
                t = int(tj[j])
                c0 = int(colbase[j])
                if t > 0:
                    nc.vector.reduce_sum(
                        out=draw[:, j : j + 1],
                        in_=wsm_all[:, c0 : c0 + t],
                        axis=mybir.AxisListType.X,
                    )
                else:
                    nc.vector.memset(draw[:, j : j + 1], 0.0)
            nc.vector.tensor_scalar_add(out=draw[:], in0=draw[:], scalar1=1.0)
            nc.scalar.activation(
                out=draw[:], in_=draw[:], func=mybir.ActivationFunctionType.Sqrt
            )
            nc.vector.reciprocal(out=dis[:], in_=draw[:])

            # ---- DRAM intermediates ----
            tables = []
            ag_ins = []
            for ell in range(3):
                ag_in = dp.tile([shard_n, CH], BF16, name=f"agin{ell}")
                table = dp.tile(
                    [NCORES * shard_n, CH], BF16, name=f"table{ell}", addr_space="Shared"
                )
                ag_ins.append(ag_in)
                tables.append(table)

            lrelu = mybir.ActivationFunctionType.Lrelu

            def emit_A(ell, j):
                hsl = h[:, j * CH : (j + 1) * CH]
                htp = pp.tile([P, P], F32, tag="htp")
                nc.tensor.transpose(out=htp[:], in_=hsl, identity=ident[:])
                hts = wk.tile([P, P], F32, tag="hts")
                nc.vector.tensor_copy(out=hts[:], in_=htp[:])
                zp = pp2.tile([P, CH], F32, tag="zp")
                nc.tensor.matmul(
                    out=zp[:], lhsT=hts[:], rhs=Wp[ell][:], start=True, stop=True
                )
                tsl = tloc[:, j * CH : (j + 1) * CH]
                nc.vector.tensor_scalar_mul(
                    out=tsl, in0=zp[:], scalar1=dis[:, j : j + 1]
                )
                stg = wk.tile([P, CH], BF16, tag="stg")
                nc.scalar.activation(
                    out=stg[:], in_=tsl, func=mybir.ActivationFunctionType.Copy
                )
                nc.sync.dma_start(out=ag_ins[ell][j * P : (j + 1) * P, :], in_=stg[:])

            def emit_AG(ell):
                nc.gpsimd.collective_compute(
                    "AllGather",
                    mybir.AluOpType.bypass,
                    replica_groups=[list(range(NCORES))],
                    ins=[ag_ins[ell][:]],
                    outs=[tables[ell][:]],
                )

            def emit_CD(ell, j):
                t = int(tj[j])
                c0 = int(colbase[j])
                acc = wk.tile([P, CH], F32, tag="acc")
                if t > 0:
                    g_t = gp.tile([P, TJMAX * 2 * CH], BF16, tag="g")
                    nc.gpsimd.dma_gather(
                        g_t[:, : t * 2 * CH].rearrange(
                            "p (t c) -> p t c", c=2 * CH
                        ),
                        tables[ell][:].rearrange("(r two) c -> r (two c)", two=2),
                        idx_all[:, 8 * c0 : 8 * (c0 + t)],
                        128 * t,
                        128 * t,
                        2 * CH,
                        elem_step=2 * CH,
                        single_packet=False,
                    )
                    # weighted multiply with parity-mask weights (2x bf16 mode)
                    gv = g_t[:, : t * 2 * CH].rearrange(
                        "p (tn h two) -> p tn h two", h=CH // 2, two=2
                    )
                    wv = (
                        wpr_all[:, 4 * c0 : 4 * (c0 + t)]
                        .rearrange("p (tn two) -> p tn two", two=2)
                        .unsqueeze(2)
                        .to_broadcast((P, 2 * t, CH // 2, 2))
                    )
                    nc.vector.tensor_tensor(
                        out=gv, in0=gv, in1=wv, op=mybir.AluOpType.mult
                    )
                    # tree-fold over 2t half-rows (contiguous bf16 adds)
                    n = 2 * t
                    while n > 2:
                        hh = n // 2
                        nc.vector.tensor_tensor(
                            out=g_t[:, : hh * CH],
                            in0=g_t[:, : hh * CH],
                            in1=g_t[:, (n - hh) * CH : n * CH],
                            op=mybir.AluOpType.add,
                        )
                        n = n - hh
                    if n == 2:
                        nc.vector.tensor_tensor(
                            out=acc[:],
                            in0=g_t[:, :CH],
                            in1=g_t[:, CH : 2 * CH],
                            op=mybir.AluOpType.add,
                        )
                    else:
                        nc.vector.tensor_copy(out=acc[:], in_=g_t[:, :CH])
                    nc.vector.tensor_add(
                        out=acc[:], in0=acc[:], in1=tloc[:, j * CH : (j + 1) * CH]
                    )
                else:
                    nc.vector.tensor_copy(
                        out=acc[:], in_=tloc[:, j * CH : (j + 1) * CH]
                    )
                # acc = acc*dis + bias   (fused)
                nc.vector.scalar_tensor_tensor(
                    out=acc[:],
                    in0=acc[:],
                    scalar=dis[:, j : j + 1],
                    in1=biasp[ell][:],
                    op0=mybir.AluOpType.mult,
                    op1=mybir.AluOpType.add,
                )
                if ell >= 1:
                    nc.vector.tensor_add(
                        out=acc[:], in0=acc[:], in1=h[:, j * CH : (j + 1) * CH]
                    )
                nc.scalar.activation(
                    out=h[:, j * CH : (j + 1) * CH], in_=acc[:], func=lrelu, alpha=0.01
                )

            def emit_head(j):
                hsl = h[:, j * CH : (j + 1) * CH]
                htp = pp.tile([P, P], F32, tag="htp")
                nc.tensor.transpose(out=htp[:], in_=hsl, identity=ident[:])
                hts = wk.tile([P, P], F32, tag="hts")
                nc.vector.tensor_copy(out=hts[:], in_=htp[:])
                z1p = pp2.tile([P, CH], F32, tag="zp")
                nc.tensor.matmul(
                    out=z1p[:], lhsT=hts[:], rhs=Wt["lin1"][:], start=True, stop=True
                )
                z1 = wk.tile([P, CH], F32, tag="z1")
                nc.vector.tensor_add(out=z1[:], in0=z1p[:], in1=vec_t["l1b"][:])
                nc.scalar.activation(out=z1[:], in_=z1[:], func=lrelu, alpha=0.01)
                z1tp = pp.tile([P, P], F32, tag="htp")
                nc.tensor.transpose(out=z1tp[:], in_=z1[:], identity=ident[:])
                z1ts = wk.tile([P, P], F32, tag="hts")
                nc.vector.tensor_copy(out=z1ts[:], in_=z1tp[:])
                z2p = pp2.tile([P, COUT], F32, tag="z2p")
                nc.tensor.matmul(
                    out=z2p[:], lhsT=z1ts[:], rhs=lin2_t[:], start=True, stop=True
                )
                yt = wk.tile([P, COUT], F32, tag="yt")
                nc.vector.tensor_add(out=yt[:], in0=z2p[:], in1=l2b_t[:])
                nc.sync.dma_start(out=y_ext[j * P : (j + 1) * P, :], in_=yt[:])

            # layer 0 table from x, then pipeline: CD(l) interleaved with A(l+1).
            # Blocks run biggest-first so the post-gather tail before each
            # AllGather is as short as possible.
            jorder = sorted(range(nblocks), key=lambda j: -int(tj[j]))
            for j in range(nblocks):
                emit_A(0, j)
            emit_AG(0)
            for ell in range(3):
                for j in jorder:
                    emit_CD(ell, j)
                    if ell < 2:
                        emit_A(ell + 1, j)
                    else:
                        emit_head(j)
                if ell < 2:
                    emit_AG(ell + 1)

    nc.compile()
    return nc


def kernel(**inputs):
    x = np.asarray(inputs["x"], np.float32)
    edge_index = np.asarray(inputs["edge_index"], np.int64)
    edge_weights = np.asarray(inputs["edge_weights"], np.float32)

    prep = _host_prep(x, edge_index, edge_weights)
    nblocks, tj, colbase, S, shard_n = (
        prep["nblocks"],
        prep["tj"],
        prep["colbase"],
        prep["S"],
        prep["shard_n"],
    )

    nc = _build_program(nblocks, tj, colbase, S, shard_n)

    rep = lambda v: np.tile(np.asarray(v, np.float32)[None, :], (P, 1))
    in_maps = []
    for c in range(NCORES):
        m = {
            "x": prep["x_sh"][c],
            "idx": prep["idx16"][c],
            "wsm": prep["w_arr"][c],
            "wpr": np.ascontiguousarray(prep["wm"][c].astype(ml_dtypes.bfloat16)),
            "w1": np.asarray(inputs["w1"], np.float32),
            "cw0": np.asarray(inputs["conv_ws"], np.float32)[0],
            "cw1": np.asarray(inputs["conv_ws"], np.float32)[1],
            "lin1": np.asarray(inputs["lin1_w"], np.float32),
            "lin2": np.asarray(inputs["lin2_w"], np.float32),
            "l1b": rep(inputs["lin1_b"]),
            "l2b": rep(inputs["lin2_b"]),
        }
        for ell in range(3):
            if ell == 0:
                g, b, mm, v = (
                    inputs["bn1_g"],
                    inputs["bn1_b"],
                    inputs["bn1_m"],
                    inputs["bn1_v"],
                )
                cb = inputs["b1"]
            else:
                g, b, mm, v = (
                    np.asarray(inputs["bns_g"])[ell - 1],
                    np.asarray(inputs["bns_b"])[ell - 1],
                    np.asarray(inputs["bns_m"])[ell - 1],
                    np.asarray(inputs["bns_v"])[ell - 1],
                )
                cb = np.asarray(inputs["conv_bs"])[ell - 1]
            m[f"bn{ell}_g"] = rep(g)
            m[f"bn{ell}_b"] = rep(b)
            m[f"bn{ell}_m"] = rep(mm)
            m[f"bn{ell}_v"] = rep(v)
            m[f"cb{ell}"] = rep(cb)
        in_maps.append(m)

    res = run_bass_kernel_spmd(nc, in_maps, core_ids=list(range(NCORES)))
    global _last_results
    _last_results = res

    out = np.empty((N, COUT), np.float32)
    order = prep["order"]
    for c in range(NCORES):
        ranks = np.arange(c, N, NCORES)
        out[order[ranks]] = res.results[c]["y"][: len(ranks)]
    return out
